# revision 4
# baseline (speedup 1.0000x reference)
"""Multi-head self-attention (B=8, S=1024, D=1024, H=16) on 8 Trainium2 cores.

Sharding: pure data-parallel over batch - core b computes attention for x[b];
weights replicated.

Per-core bf16 pipeline (PSUM accumulation stays fp32):
  - gpsimd casting DMAs load x/W row-blocks from HBM fp32 directly into SBUF
    bf16 stage tiles; PE transpose-mode matmuls (bf16: 128 cyc per 128x128
    tile, 4 tiles batched into one PSUM bank via start/stop flags) build
    feature-major X^T / W^T catalogs; batched [128,512] DVE/gpsimd evictions.
  - Q^T/K^T [d, s] = W^T-stationary matmuls with 512-wide X^T moving slabs;
    V [s, d] = X^T-stationary matmuls with 512-wide Wv^T moving slabs.
    Q/K bias+scale fused into the DVE eviction (tensor_scalar); V bias added
    during eviction from a partition-broadcast copy of bv (no bias matmuls).
  - scores^T for a head pair go into one [128, 1024] PSUM tile ([A | B]) so a
    single ACT exp instruction covers both heads; exp writes bf16.  No row-max
    subtraction (scores ~ N(0,1)).  PV accumulates ctx^T[65, 512] over s_k;
    VA carries a ones column so row 64 is the softmax denominator.
  - normalize: DVE reciprocal + gpsimd partition_broadcast + DVE multiply
    (the PSUM ctx bank is evicted raw first so the next chunk isn't gated).
  - output: DMA-transpose ctx^T -> ctx [q, f] bf16, DVE cast to fp32, DMA out;
    the very last chunk instead PE-transposes raw ctx+denominator and divides
    with a per-q-partition scalar for the shortest possible tail.

Software pipeline: weight casts run well ahead on the gpsimd queue; the
transpose / projection / V-projection PE work for later pairs is chopped into
<=1.7us units emitted between attention sk-steps so the PE never starves
while ACT crunches exp.  matmul start=True zeroes the whole 2KB PSUM bank, so
every accumulation/transpose group writes full-bank regions.
"""

import numpy as np

import concourse.bacc as bacc
import concourse.mybir as mybir
import concourse.tile as tile
from concourse.bass_utils import run_bass_kernel_spmd
from concourse.masks import make_identity

B = 8
S = 1024
D = 1024
H = 16
HD = 64
P = 128
NT = D // P          # 8 tiles along d / k / s
CH = 512             # s_q / dv chunk
NCH = S // CH        # 2 chunks
SCALE = float(HD) ** -0.5

F32 = mybir.dt.float32
BF16 = mybir.dt.bfloat16
MULT = mybir.AluOpType.mult
ADD = mybir.AluOpType.add
EXP = mybir.ActivationFunctionType.Exp


def _build():
    nc = bacc.Bacc("TRN2", target_bir_lowering=False, debug=False, num_devices=B)

    x = nc.dram_tensor("x", [S, D], F32, kind="ExternalInput")
    wq = nc.dram_tensor("wq", [D, D], F32, kind="ExternalInput")
    wk = nc.dram_tensor("wk", [D, D], F32, kind="ExternalInput")
    wv = nc.dram_tensor("wv", [D, D], F32, kind="ExternalInput")
    bq = nc.dram_tensor("bq", [D], F32, kind="ExternalInput")
    bk = nc.dram_tensor("bk", [D], F32, kind="ExternalInput")
    bv = nc.dram_tensor("bv", [D], F32, kind="ExternalInput")
    out = nc.dram_tensor("out", [S, D], F32, kind="ExternalOutput")

    with nc.allow_low_precision("bf16 matmul pipeline"), tile.TileContext(nc) as tc:
        with (
            tc.tile_pool(name="consts", bufs=1) as consts,
            tc.tile_pool(name="persist", bufs=1) as persist,
            tc.tile_pool(name="xst", bufs=8) as xstage,
            tc.tile_pool(name="wst", bufs=12) as wstage,
            tc.tile_pool(name="qkp", bufs=3) as qk_pool,
            tc.tile_pool(name="ptp", bufs=4) as pt_pool,
            tc.tile_pool(name="ctxp", bufs=2) as ctx_pool,
            tc.tile_pool(name="ocp", bufs=3) as oc_pool,
            tc.tile_pool(name="rvp", bufs=3) as rv_pool,
            tc.tile_pool(name="psum", bufs=1, space="PSUM") as psum,
        ):
            # ---- constants / biases ----
            identf = consts.tile([P, P], F32, name="identf")
            make_identity(nc, identf)
            ident = consts.tile([P, P], BF16, name="ident")
            nc.vector.tensor_copy(out=ident, in_=identf)
            bqs = consts.tile([P, NT], F32, name="bqs")
            nc.sync.dma_start(out=bqs, in_=bq[:].rearrange("(j p) -> p j", p=P))
            bqss = consts.tile([P, NT], F32, name="bqss")
            nc.vector.tensor_scalar_mul(bqss, bqs, SCALE)
            bks = consts.tile([P, NT], F32, name="bks")
            nc.sync.dma_start(out=bks, in_=bk[:].rearrange("(j p) -> p j", p=P))
            bvs = consts.tile([1, D], F32, name="bvs")
            nc.sync.dma_start(out=bvs, in_=bv[:].rearrange("(o d) -> o d", o=1))
            bvb = consts.tile([P, D], F32, name="bvb")

            # feature-major catalogs: CAT[p, j, f] = src[f', j*128+p] layouts
            XTc = persist.tile([P, NT, S], BF16, name="xtc", tag="xtc")
            WQc = persist.tile([P, NT, S], BF16, name="wqc", tag="wqc")
            WKc = persist.tile([P, NT, S], BF16, name="wkc", tag="wkc")
            WVc = persist.tile([P, NT, S], BF16, name="wvc", tag="wvc")
            # V + ones column: VA[st][p_s, h, f]  (f==64 is the ones column)
            VA = [persist.tile([P, H, HD + 1], BF16, name=f"va{st}", tag=f"va{st}")
                  for st in range(NT)]

            evict_flip = [0]

            def cast_x(st):
                xs = xstage.tile([P, S], BF16, name=f"xs{st}", tag="xs")
                nc.gpsimd.dma_start(out=xs, in_=x[st * P:(st + 1) * P, :])
                return xs

            def cast_w(dram, blk, nm):
                ws = wstage.tile([P, S], BF16, name=f"ws_{nm}{blk}", tag="ws")
                nc.gpsimd.dma_start(out=ws, in_=dram[blk * P:(blk + 1) * P, :])
                return ws

            def pose(src, cat, blk):
                """Transpose stage tile src (row-block blk) into catalog cat.

                cat[:, j, blk*128:(blk+1)*128] = src[:, j*128:(j+1)*128].T
                Two groups of 4 tiles, each batched into one PSUM bank with a
                single DVE eviction.
                """
                for g in range(2):
                    ps = psum.tile([P, CH], BF16, tag="work", bufs=2,
                                   name=f"tp{blk}_{g}")
                    for i in range(4):
                        j = 4 * g + i
                        nc.tensor.matmul(
                            ps[:, i * P:(i + 1) * P],
                            lhsT=src[:, j * P:(j + 1) * P], rhs=ident,
                            is_transpose=True, start=(i == 0), stop=(i == 3),
                        )
                    dst = cat[:, 4 * g:4 * g + 4, blk * P:(blk + 1) * P]
                    srcap = ps.rearrange("p (j f) -> p j f", j=4)
                    # always DVE: a gpsimd eviction would queue behind cast
                    # preps on the Pool engine and stall the PSUM work-bank
                    # rotation (and with it the PE)
                    nc.vector.tensor_copy(out=dst, in_=srcap)

            # ---- projection / V units (each ~1.7us of PE) ----
            qk_tiles = {}

            def proj_unit(jp, which, c):
                qt, kt_t = qk_tiles[jp]
                Wc = WQc if which == "q" else WKc
                def w_sl(kt):
                    return Wc[:, kt, jp * P:(jp + 1) * P]
                ps = psum.tile([P, CH], F32, tag="work", bufs=2,
                               name=f"ps{which}{jp}_{c}")
                for kt in range(NT):
                    nc.tensor.matmul(
                        ps, lhsT=w_sl(kt),
                        rhs=XTc[:, kt, c * CH:(c + 1) * CH],
                        start=(kt == 0), stop=(kt == NT - 1),
                    )
                if which == "q":
                    nc.vector.tensor_scalar(
                        out=qt[:, c * CH:(c + 1) * CH], in0=ps, scalar1=SCALE,
                        scalar2=bqss[:, jp:jp + 1], op0=MULT, op1=ADD,
                    )
                else:
                    nc.vector.tensor_scalar(
                        out=kt_t[:, c * CH:(c + 1) * CH], in0=ps,
                        scalar1=bks[:, jp:jp + 1], scalar2=None, op0=ADD,
                    )

            def v_unit(st, ch):
                """V projection for heads ch*8..ch*8+7 on s-tile st."""
                ps = psum.tile([P, CH], F32, tag="work", bufs=2,
                               name=f"psv{st}_{ch}")
                for kt in range(NT):
                    nc.tensor.matmul(
                        ps, lhsT=XTc[:, kt, st * P:(st + 1) * P],
                        rhs=WVc[:, kt, ch * CH:(ch + 1) * CH],
                        start=(kt == 0), stop=(kt == NT - 1),
                    )
                nc.vector.tensor_tensor(
                    out=VA[st][:, ch * 8:(ch + 1) * 8, 0:HD],
                    in0=ps.rearrange("p (h f) -> p h f", h=8),
                    in1=bvb[:, ch * CH:(ch + 1) * CH]
                    .rearrange("p (h f) -> p h f", h=8),
                    op=ADD,
                )

            def new_qk(jp):
                qk_tiles[jp] = (
                    qk_pool.tile([P, S], BF16, name=f"qt{jp}", tag="qt"),
                    qk_pool.tile([P, S], BF16, name=f"kt{jp}", tag="kt"),
                )

            # ---- attention for one head pair ----
            def attn(jp, fillers, last=False):
                qt, kt_t = qk_tiles[jp]
                ctx_t = ctx_pool.tile([P, S], BF16, name=f"ctx{jp}", tag="ctxt")
                hA, hB = 2 * jp, 2 * jp + 1
                ctx_ps = {}
                pv_q = []

                def emit_pv(c, sk, ptab):
                    psA, psB = ctx_ps[c]
                    nc.tensor.matmul(
                        psA, lhsT=VA[sk][:, hA, :], rhs=ptab[:, 0:CH],
                        start=(sk == 0), stop=(sk == NT - 1),
                    )
                    nc.tensor.matmul(
                        psB, lhsT=VA[sk][:, hB, :], rhs=ptab[:, CH:2 * CH],
                        start=(sk == 0), stop=(sk == NT - 1),
                    )
                    if sk == NT - 1:
                        normalize(c)

                def normalize(c):
                    psA, psB = ctx_ps.pop(c)
                    if last and c == NCH - 1:
                        # final chunk: transpose RAW ctx+denominator via PE
                        # (idle by now) into q-major mini-tiles, then divide
                        # with a per-q-partition scalar - shortest possible
                        # tail chain
                        ocf = oc_pool.tile([P, 4, P], F32, name=f"ocf{jp}_{c}",
                                           tag="ocf")
                        for half, psX in ((0, psA), (1, psB)):
                            crh = rv_pool.tile([HD + 1, CH], BF16,
                                               name=f"crb{half}", tag="crb")
                            nc.vector.tensor_copy(out=crh, in_=psX)
                            ptb = psum.tile([P, 4, P], BF16, tag="work",
                                            bufs=2, name=f"ptb{half}")
                            for i in range(4):
                                nc.tensor.matmul(
                                    ptb[:, i, :], lhsT=crh[:, i * P:(i + 1) * P],
                                    rhs=ident[0:HD + 1, :],
                                    is_transpose=True,
                                    start=(i == 0), stop=(i == 3),
                                )
                            for i in range(4):
                                rv8 = rv_pool.tile([P, 1], F32,
                                                   name=f"rv8{half}{i}",
                                                   tag="rv8")
                                nc.vector.reciprocal(out=rv8,
                                                     in_=ptb[:, i, HD:HD + 1])
                                nc.vector.tensor_scalar(
                                    out=ocf[:, i, half * HD:(half + 1) * HD],
                                    in0=ptb[:, i, 0:HD], scalar1=rv8,
                                    scalar2=None, op0=MULT,
                                )
                        nc.sync.dma_start(
                            out=out[c * CH:(c + 1) * CH, jp * P:(jp + 1) * P]
                            .rearrange("(st p) f -> p st f", p=P),
                            in_=ocf,
                        )
                        return
                    for half, psX in ((0, psA), (1, psB)):
                        # evict raw ctx+denominator first: frees the PSUM ctx
                        # bank for the next chunk, normalize runs off-path.
                        # (skipped for the last pair: the extra hop would
                        # lengthen the output tail)
                        if not last:
                            cr = rv_pool.tile([HD + 1, CH], F32,
                                              name=f"cr{jp}{c}{half}", tag="cr")
                            nc.vector.tensor_copy(out=cr, in_=psX)
                        else:
                            cr = psX
                        rv = rv_pool.tile([1, CH], F32, name=f"rv{jp}{c}{half}",
                                          tag="rv")
                        nc.vector.reciprocal(out=rv, in_=cr[HD:HD + 1, :])
                        bc = rv_pool.tile([HD, CH], F32, name=f"bc{jp}{c}{half}",
                                          tag="bc")
                        nc.gpsimd.partition_broadcast(bc, rv)
                        nc.vector.tensor_mul(
                            out=ctx_t[half * HD:(half + 1) * HD, c * CH:(c + 1) * CH],
                            in0=cr[0:HD, :], in1=bc,
                        )
                    # output chunk c: ctx^T -> ctx via DMA transpose, cast, store
                    occ = oc_pool.tile([P, 4, P], BF16, name=f"oc{jp}_{c}",
                                       tag="oc")
                    nc.sync.dma_start_transpose(occ, ctx_t[:, c * CH:(c + 1) * CH])
                    ocf = oc_pool.tile([P, 4, P], F32, name=f"ocf{jp}_{c}",
                                       tag="ocf")
                    nc.vector.tensor_copy(out=ocf, in_=occ)
                    nc.sync.dma_start(
                        out=out[c * CH:(c + 1) * CH, jp * P:(jp + 1) * P]
                        .rearrange("(st p) f -> p st f", p=P),
                        in_=ocf,
                    )

                # scores/exp run one step ahead of the PVs so the ~1.4us
                # exp+sem latency never blocks the PE
                for c in range(NCH):
                    ctx_ps[c] = (
                        psum.tile([HD + 1, CH], F32, tag="ctx", bufs=2,
                                  name=f"cA{jp}_{c}"),
                        psum.tile([HD + 1, CH], F32, tag="ctx", bufs=2,
                                  name=f"cB{jp}_{c}"),
                    )
                    for sk in range(NT):
                        ps_s = psum.tile([P, 2 * CH], F32, tag="sc", bufs=2,
                                         name=f"ss{jp}_{c}_{sk}")
                        nc.tensor.matmul(
                            ps_s[:, 0:CH], lhsT=kt_t[0:HD, sk * P:(sk + 1) * P],
                            rhs=qt[0:HD, c * CH:(c + 1) * CH],
                            start=True, stop=True, tile_position=(0, 0),
                        )
                        nc.tensor.matmul(
                            ps_s[:, CH:2 * CH], lhsT=kt_t[HD:P, sk * P:(sk + 1) * P],
                            rhs=qt[HD:P, c * CH:(c + 1) * CH],
                            start=True, stop=True, tile_position=(HD, 0),
                        )
                        ptab = pt_pool.tile([P, 2 * CH], BF16,
                                            name=f"pt{jp}_{c}_{sk}", tag="pt")
                        nc.scalar.activation(out=ptab, in_=ps_s, func=EXP)
                        pv_q.append((c, sk, ptab))
                        if len(pv_q) >= 2:
                            # fillers before the PVs: safe spot for v_units
                            # that later sk-steps of this pair depend on
                            for _ in range(2 if len(fillers) > 8 else 1):
                                if fillers:
                                    fillers.pop(0)()
                            emit_pv(*pv_q.pop(0))
                while pv_q:
                    if fillers:
                        fillers.pop(0)()
                    emit_pv(*pv_q.pop(0))
                while fillers:
                    fillers.pop(0)()

            # ---- emission schedule ----
            # gpsimd (Pool) cast order: x first, then W in need order; the
            # first v-eviction needs bvb, so bvs-broadcast rides between.
            # PE consumes stage tiles via pose() immediately after each cast.
            xs_t = {}
            ws_t = {}
            for st in range(4):
                xs_t[st] = cast_x(st)
            ws_t["q0"] = cast_w(wq, 0, "q")
            ws_t["k0"] = cast_w(wk, 0, "k")
            for st in range(4, 8):
                xs_t[st] = cast_x(st)
            ws_t["v0"] = cast_w(wv, 0, "v")
            nc.gpsimd.partition_broadcast(bvb, bvs)
            ws_t["v1"] = cast_w(wv, 1, "v")
            ws_t["v2"] = cast_w(wv, 2, "v")
            ws_t["v3"] = cast_w(wv, 3, "v")
            ws_t["q1"] = cast_w(wq, 1, "q")
            ws_t["k1"] = cast_w(wk, 1, "k")
            for r in range(4, 8):
                ws_t[f"v{r}"] = cast_w(wv, r, "v")
            ws_t["q2"] = cast_w(wq, 2, "q")
            ws_t["k2"] = cast_w(wk, 2, "k")
            for j in range(3, NT):
                ws_t[f"q{j}"] = cast_w(wq, j, "q")
                ws_t[f"k{j}"] = cast_w(wk, j, "k")
            for st in range(NT):
                nc.vector.memset(VA[st][:, :, HD:HD + 1], 1.0)

            # PE: startup poses + first projections (chunk 0 only needs
            # x-tiles 0..3, so project before posing x4..7)
            for st in range(4):
                pose(xs_t[st], XTc, st)
            pose(ws_t["q0"], WQc, 0)
            pose(ws_t["k0"], WKc, 0)
            new_qk(0)
            proj_unit(0, "q", 0)
            proj_unit(0, "k", 0)
            for st in range(4, 8):
                pose(xs_t[st], XTc, st)
            for r in range(4):
                pose(ws_t[f"v{r}"], WVc, r)

            def mk_proj(jp, which, c):
                return lambda: proj_unit(jp, which, c)

            def mk_v(st, ch):
                return lambda: v_unit(st, ch)

            def mk_pose(key, cat, blk):
                return lambda: pose(ws_t[key], cat, blk)

            for jp in range(NT):
                fillers = []
                if jp == 0:
                    # v(st) must land before PV(sk=st); k/q chunk-1 before the
                    # sk>=4 scores / c=1 steps
                    fillers += [mk_v(0, 0), mk_proj(0, "k", 1),
                                mk_v(1, 0), mk_proj(0, "q", 1),
                                mk_v(2, 0), mk_v(3, 0),
                                mk_v(4, 0), mk_v(5, 0), mk_v(6, 0), mk_v(7, 0)]
                else:
                    # each pair projects its own chunk 1 (k-c1 consumed from
                    # step 4, q-c1 from step 8) - keeps PE filler work
                    # available even for the last pair
                    fillers += [mk_proj(jp, "k", 1), mk_proj(jp, "q", 1)]
                if jp + 1 < NT:
                    new_qk(jp + 1)
                    fillers += [mk_pose(f"q{jp + 1}", WQc, jp + 1),
                                mk_pose(f"k{jp + 1}", WKc, jp + 1),
                                mk_proj(jp + 1, "q", 0),
                                mk_proj(jp + 1, "k", 0)]
                if jp == 0:
                    for r in range(4, 8):
                        fillers.append(mk_pose(f"v{r}", WVc, r))
                if jp in (1, 2, 3):
                    # ch1 v-tiles must all land before attn(4)
                    for st in range((jp - 1) * 3, min(NT, jp * 3)):
                        fillers.append(mk_v(st, 1))
                attn(jp, fillers, last=(jp == NT - 1))

    nc.compile()
    return nc


_NC = None


def _get_nc():
    global _NC
    if _NC is None:
        _NC = _build()
    return _NC


def kernel(x, Wq, Wk, Wv, bq, bk, bv):
    x = np.ascontiguousarray(np.asarray(x, dtype=np.float32))
    Wq = np.ascontiguousarray(np.asarray(Wq, dtype=np.float32))
    Wk = np.ascontiguousarray(np.asarray(Wk, dtype=np.float32))
    Wv = np.ascontiguousarray(np.asarray(Wv, dtype=np.float32))
    bq = np.ascontiguousarray(np.asarray(bq, dtype=np.float32))
    bk = np.ascontiguousarray(np.asarray(bk, dtype=np.float32))
    bv = np.ascontiguousarray(np.asarray(bv, dtype=np.float32))

    nc = _get_nc()
    in_maps = [
        {"x": np.ascontiguousarray(x[b]), "wq": Wq, "wk": Wk, "wv": Wv,
         "bq": bq, "bk": bk, "bv": bv}
        for b in range(B)
    ]
    res = run_bass_kernel_spmd(nc, in_maps, core_ids=list(range(B)))
    return np.stack([res.results[b]["out"] for b in range(B)], axis=0)
